# revision 1
# baseline (speedup 1.0000x reference)
"""DMPNN message-passing kernel for 8 Trainium2 NeuronCores (Bass/Tile).

Strategy (edge/data parallel, per the sharding hint):
  - Edges come in reverse pairs (2k, 2k+1).  Pairs are sharded across the 8
    cores, and each core splits its pairs into two parallel arrays hE (even
    edge of the pair) and hO (odd edge).  rev(e) is then simply "the same
    row of the sibling array" -- no shuffle ever needed.
  - Per core, pairs are sorted into 4 segments by (dst-half, src-half) of
    the pair so that every dma_gather / dma_scatter_add instruction indexes
    a single node-table half of <= 32768 rows (int16 index limit).
  - h is stored feature-major [128, NP] in DRAM; the per-edge matmul uses a
    stationary Wu.  PE transposes convert between edge-major gather/scatter
    tiles and feature-major compute slabs.
  - segment_sum over edge_dst = local f32 dma_scatter_add into a partial
    node table, followed by an AllReduce over the 8 cores each step.
"""
import sys

sys.path.insert(0, "/opt/trn_rl_repo")

import numpy as np

N_CORES = 8
D = 128
DE = 32
STEPS = 4
SLAB = 512          # pairs per compute slab (one PSUM bank at f32)
GROUP = 2048        # pairs per gather/scatter instruction
UNROLL = 2          # For_i body unroll
HB = 63 * 512       # node half boundary (rows 0..HB-1 in LO table)
PAD_HI = 512        # LO table padding so HI_BASE is 512-aligned


def _ceil(x, m):
    return (x + m - 1) // m * m


def _window_assign(s, d, group, node_max, max_win=192):
    """Assign each pair to a window of size `group` such that within every
    window all d values are distinct and all s values are distinct (the
    dma_scatter_add engine-race constraint).  Greedy rounds, vectorized.
    Returns (win_id array, n_windows)."""
    n = s.size
    win = np.full(n, -1, np.int32)
    used_s = np.zeros((node_max, max_win), bool)
    used_d = np.zeros((node_max, max_win), bool)
    full = np.zeros(max_win, bool)
    cnt = np.zeros(max_win, np.int64)
    rem = np.arange(n)
    while rem.size:
        free = ~(used_s[s[rem]] | used_d[d[rem]] | full[None, :])
        assert free.any(axis=1).all(), "window assigner ran out of windows"
        w = np.argmax(free, axis=1).astype(np.int64)
        # order candidates by (w, position); accept per window: unique d,
        # unique s, within remaining capacity
        order = np.lexsort((rem, w))
        ws, rs = w[order], rem[order]
        ds_, ss_ = d[rs], s[rs]
        kd = ws * np.int64(node_max) + ds_
        ks = ws * np.int64(node_max) + ss_
        first_d = np.zeros(ws.size, bool)
        first_s = np.zeros(ws.size, bool)
        od = np.lexsort((np.arange(ws.size), kd))
        first_d[od[np.concatenate(([True], kd[od][1:] != kd[od][:-1]))]] = True
        os_ = np.lexsort((np.arange(ws.size), ks))
        first_s[os_[np.concatenate(([True], ks[os_][1:] != ks[os_][:-1]))]] = True
        # capacity: rank within window among candidates
        uw, st, cts = np.unique(ws, return_index=True, return_counts=True)
        rank = np.arange(ws.size) - np.repeat(st, cts)
        ok = first_d & first_s & (rank < np.repeat(group - cnt[uw], cts))
        acc = rs[ok]
        wacc = ws[ok]
        win[acc] = wacc
        used_d[d[acc], wacc] = True
        used_s[s[acc], wacc] = True
        np.add.at(cnt, wacc, 1)
        full = cnt >= group
        rem = rem[win[rem] < 0]
    return win, (int(cnt.nonzero()[0].max()) + 1) if n else 0


def _prep(node_feature, edge_feature, edge_src, edge_dst,
          n_cores=N_CORES, group=GROUP, unroll=UNROLL, hb=HB):
    node_feature = np.asarray(node_feature, np.float32)
    edge_feature = np.asarray(edge_feature, np.float32)
    edge_src = np.asarray(edge_src)
    edge_dst = np.asarray(edge_dst)
    N = node_feature.shape[0]
    E = edge_src.shape[0]
    P = E // 2
    assert P % n_cores == 0
    per = P // n_cores

    s_all = edge_src[0::2].astype(np.int64)
    d_all = edge_dst[0::2].astype(np.int64)
    efE_all = edge_feature[0::2]
    efO_all = edge_feature[1::2]

    HI_BASE = hb + PAD_HI
    HI_N = N - hb
    LO_TRASH = hb            # idx within LO slice
    HI_TRASH = HI_N          # idx within HI slice
    OUT_PAD = _ceil(N, SLAB)
    NTAB = _ceil(max(HI_BASE + HI_N + 1, OUT_PAD + PAD_HI), 512)

    nfp = np.zeros((NTAB, D), np.float32)
    nfp[0:hb] = node_feature[0:hb]
    nfp[HI_BASE:HI_BASE + HI_N] = node_feature[hb:N]

    # Per (core, segment): assign pairs to windows of `group` so every
    # window has distinct d and distinct s (scatter engine-race rule),
    # then order by (window, s).  Window counts are leveled across cores
    # (SPMD: identical program, data-only differences).
    cores = []
    nwin = np.zeros((n_cores, 4), np.int64)
    for c in range(n_cores):
        sl = slice(c * per, (c + 1) * per)
        sc, dc = s_all[sl], d_all[sl]
        a = (dc >= hb).astype(np.int64)
        b = (sc >= hb).astype(np.int64)
        seg = a * 2 + b
        per_seg = []
        for g in range(4):
            m = np.flatnonzero(seg == g)
            s_m, d_m = sc[m], dc[m]
            degd = np.bincount(d_m, minlength=N)
            degs = np.bincount(s_m, minlength=N)
            prio = np.argsort(-(degd[d_m] + degs[s_m]), kind="stable")
            win_p, nw = _window_assign(s_m[prio], d_m[prio], group, N)
            win = np.empty_like(win_p)
            win[prio] = win_p
            key = np.lexsort((s_m, win))
            per_seg.append((m[key], win[key], nw))
            nwin[c, g] = nw
        cores.append((sc, dc, efE_all[sl], efO_all[sl], per_seg))
    gchunk = group * unroll
    seg_nw = [int(_ceil(max(int(nwin[:, g].max()), 1) * group, gchunk)) // group
              for g in range(4)]
    seg_sz = [nw * group for nw in seg_nw]
    NP_ = int(sum(seg_sz))
    seg_start = [0, seg_sz[0], seg_sz[0] + seg_sz[1],
                 seg_sz[0] + seg_sz[1] + seg_sz[2]]

    shards = []
    for c in range(n_cores):
        sc, dc, efE_c, efO_c, per_seg = cores[c]
        sIdx = np.zeros(NP_, np.int64)
        dIdx = np.zeros(NP_, np.int64)
        efE_p = np.zeros((NP_, DE), np.float32)
        efO_p = np.zeros((NP_, DE), np.float32)
        for g in range(4):
            a, b = g // 2, g % 2
            st = seg_start[g]
            s_tr = LO_TRASH if b == 0 else HI_TRASH
            d_tr = LO_TRASH if a == 0 else HI_TRASH
            sIdx[st:st + seg_sz[g]] = s_tr
            dIdx[st:st + seg_sz[g]] = d_tr
            order, wins, nw = per_seg[g]
            # window w occupies [st + w*group, st + (w+1)*group); place the
            # window's pairs at the front of its span
            if order.size:
                counts = np.bincount(wins, minlength=nw)
                assert counts.max() <= group
                starts = np.concatenate(([0], np.cumsum(counts)))[:-1]
                rank = np.arange(order.size) - starts[wins]
                pos = st + wins * group + rank
                sIdx[pos] = sc[order] - hb * b
                dIdx[pos] = dc[order] - hb * a
                efE_p[pos] = efE_c[order]
                efO_p[pos] = efO_c[order]
        assert sIdx.max() < 32768 and dIdx.max() < 32768

        def wrap16(v):
            t = v.astype(np.int16).reshape(-1, 16).T       # [16, NP/16]
            return np.ascontiguousarray(np.tile(t, (8, 1)))  # [128, NP/16]

        shards.append({
            "nfp": nfp,
            "efE": np.ascontiguousarray(efE_p.T),
            "efO": np.ascontiguousarray(efO_p.T),
            "sIdx": wrap16(sIdx),
            "dIdx": wrap16(dIdx),
        })

    meta = dict(N=N, NP=NP_, NTAB=NTAB, HI_BASE=HI_BASE, OUT_PAD=OUT_PAD,
                seg_sz=seg_sz, seg_start=seg_start, HB=hb,
                n_cores=n_cores, group=group, unroll=unroll)
    return shards, meta


def _build(meta):
    import concourse.bass as bass
    import concourse.tile as tile
    from concourse import bacc, mybir

    f32 = mybir.dt.float32
    i16 = mybir.dt.int16
    NP_ = meta["NP"]
    NTAB = meta["NTAB"]
    HI_BASE = meta["HI_BASE"]
    OUT_PAD = meta["OUT_PAD"]
    group = meta["group"]
    unroll = meta["unroll"]
    n_cores = meta["n_cores"]
    hb = meta["HB"]
    jb = hb // SLAB          # final-pass boundary slab

    nc = bacc.Bacc("TRN2", target_bir_lowering=False, debug=False,
                   enable_asserts=False, num_devices=n_cores)

    nfp_t = nc.dram_tensor("nfp", [NTAB, D], f32, kind="ExternalInput")
    efE_t = nc.dram_tensor("efE", [DE, NP_], f32, kind="ExternalInput")
    efO_t = nc.dram_tensor("efO", [DE, NP_], f32, kind="ExternalInput")
    sIdx_t = nc.dram_tensor("sIdx", [128, NP_ // 16], i16, kind="ExternalInput")
    dIdx_t = nc.dram_tensor("dIdx", [128, NP_ // 16], i16, kind="ExternalInput")
    Wi_t = nc.dram_tensor("Wi", [D + DE, D], f32, kind="ExternalInput")
    Wu_t = nc.dram_tensor("Wu", [D, D], f32, kind="ExternalInput")
    Wf_t = nc.dram_tensor("Wf", [2 * D, D], f32, kind="ExternalInput")
    bi_t = nc.dram_tensor("bi", [D, 1], f32, kind="ExternalInput")
    bu_t = nc.dram_tensor("bu", [D, 1], f32, kind="ExternalInput")
    bf4_t = nc.dram_tensor("bf4", [D, SLAB], f32, kind="ExternalInput")
    id_t = nc.dram_tensor("ident", [D, D], f32, kind="ExternalInput")
    out_t = nc.dram_tensor("out", [OUT_PAD, D], f32, kind="ExternalOutput")

    GP = group // 128        # em blocks per group
    GS = group // SLAB       # slabs per group

    with tile.TileContext(nc) as tc:
        with (
            tc.tile_pool(name="const", bufs=1) as constp,
            tc.tile_pool(name="work", bufs=2) as work,
            tc.tile_pool(name="emb", bufs=2) as emb,
            tc.tile_pool(name="psA", bufs=1, space="PSUM") as psA,
            tc.tile_pool(name="psB", bufs=1, space="PSUM") as psB,
            tc.tile_pool(name="dram", bufs=1, space="DRAM") as dram,
        ):
            # ---- constants in SBUF ----
            def const_load(name, shape, src_ap):
                t = constp.tile(shape, f32, tag=name)
                nc.sync.dma_start(t[:], src_ap)
                return t

            WiA = const_load("WiA", [D, D], Wi_t.ap()[0:D, :])
            WiB = const_load("WiB", [DE, D], Wi_t.ap()[D:D + DE, :])
            Wu_sb = const_load("Wu", [D, D], Wu_t.ap())
            WfA = const_load("WfA", [D, D], Wf_t.ap()[0:D, :])
            WfB = const_load("WfB", [D, D], Wf_t.ap()[D:2 * D, :])
            bi_sb = const_load("bi", [D, 1], bi_t.ap())
            bu_sb = const_load("bu", [D, 1], bu_t.ap())
            bf4_sb = const_load("bf4", [D, SLAB], bf4_t.ap())
            id_sb = const_load("ident", [D, D], id_t.ap())
            sIdx_sb = constp.tile([128, NP_ // 16], i16, tag="sIdx")
            nc.sync.dma_start(sIdx_sb[:], sIdx_t.ap())
            dIdx_sb = constp.tile([128, NP_ // 16], i16, tag="dIdx")
            nc.sync.dma_start(dIdx_sb[:], dIdx_t.ap())
            zero_sb = constp.tile([128, 2048], f32, tag="zero")
            nc.vector.memset(zero_sb[:], 0.0)

            # ---- DRAM state ----
            hE = [dram.tile([D, NP_], f32, name=f"hE{k}", tag=f"hE{k}")
                  for k in range(2)]
            hO = [dram.tile([D, NP_], f32, name=f"hO{k}", tag=f"hO{k}")
                  for k in range(2)]
            agg = [dram.tile([NTAB, D], f32, name=f"agg{k}", tag=f"agg{k}")
                   for k in range(3)]

            def tab_slice(t, hi):
                ap = t[:] if hasattr(t, "opt") else t.ap()
                return ap[HI_BASE:NTAB, :] if hi else ap[0:HI_BASE, :]

            def zero_table(t):
                zr = zero_sb[:].rearrange("p (a f) -> p a f", f=D)
                r0 = 0
                while r0 < NTAB:
                    zc = 2048 if NTAB - r0 >= 2048 else 512
                    nc.sync.dma_start(
                        t[:][r0:r0 + zc, :].rearrange("(a p) f -> p a f", p=128),
                        zr[:, :zc // 128, :],
                    )
                    r0 += zc

            def idx_slice(which, i, off):
                sb = sIdx_sb if which == "s" else dIdx_sb
                return sb[:, bass.ds(i * (group // 16) + off // 16, group // 16)]

            def transpose_to_fm(dst_ps, src_em, nblk, sub):
                # src_em [128, GP, D] block range -> dst_ps [D, nblk*128]
                for blk in range(nblk):
                    nc.tensor.transpose(
                        dst_ps[:, blk * 128:(blk + 1) * 128],
                        src_em[:, sub * nblk + blk, :], id_sb[:])

            def transpose_to_em(dst_ps, src_fm, nblk):
                # src_fm [D, nblk*128] -> dst_ps em blocks [128, nblk, D]
                for blk in range(nblk):
                    nc.tensor.transpose(
                        dst_ps[:, blk, :],
                        src_fm[:, blk * 128:(blk + 1) * 128], id_sb[:])

            NB = SLAB // 128

            def body(kind, seg, i, src, tgt, hin, hout):
                """One group of `group` pairs for segment seg at group idx i.

                kind: "init" or "step".  src/tgt: agg tables (src None for
                init gathers which read nfp).  hin/hout: (hE, hO) pairs.
                """
                a, b = seg // 2, seg % 2
                off = meta["seg_start"][seg]
                gtab = nfp_t if kind == "init" else src
                AE_em = emb.tile([128, GP, D], f32, tag="AE")
                nc.gpsimd.dma_gather(
                    AE_em[:], tab_slice(gtab, b), idx_slice("s", i, off),
                    num_idxs=group, num_idxs_reg=group, elem_size=D,
                    single_packet=False)
                AO_em = emb.tile([128, GP, D], f32, tag="AO")
                nc.gpsimd.dma_gather(
                    AO_em[:], tab_slice(gtab, a), idx_slice("d", i, off),
                    num_idxs=group, num_idxs_reg=group, elem_size=D,
                    single_packet=False)

                if kind == "init":
                    xE = work.tile([DE, group], f32, tag="xE")
                    nc.sync.dma_start(xE[:], efE_t.ap()[:, bass.ds(i * group + off, group)])
                    xO = work.tile([DE, group], f32, tag="xO")
                    nc.sync.dma_start(xO[:], efO_t.ap()[:, bass.ds(i * group + off, group)])
                else:
                    xE = work.tile([D, group], f32, tag="xE")
                    nc.sync.dma_start(xE[:], hin[0][:][:, bass.ds(i * group + off, group)])
                    xO = work.tile([D, group], f32, tag="xO")
                    nc.sync.dma_start(xO[:], hin[1][:][:, bass.ds(i * group + off, group)])

                emE = emb.tile([128, GP, D], f32, tag="emE")
                emO = emb.tile([128, GP, D], f32, tag="emO")
                for sub in range(GS):
                    c0 = sub * SLAB
                    for arr in range(2):
                        A_em = (AE_em, AO_em)[arr]
                        ps_a = (psA if arr == 0 else psB).tile([D, SLAB], f32, tag="afm")
                        transpose_to_fm(ps_a, A_em, NB, sub)
                        a_fm = work.tile([D, SLAB], f32, tag=f"afm{arr}")
                        nc.scalar.copy(a_fm[:], ps_a[:])
                        ps_u = (psA if arr == 0 else psB).tile([D, SLAB], f32, tag="u")
                        if kind == "init":
                            x_fm = (xE, xO)[arr]
                            nc.tensor.matmul(ps_u[:], WiA[:], a_fm[:],
                                             start=True, stop=False)
                            nc.tensor.matmul(ps_u[:], WiB[:],
                                             x_fm[:, c0:c0 + SLAB],
                                             start=False, stop=True)
                            h_t = work.tile([D, SLAB], f32, tag=f"ht{arr}")
                            nc.scalar.activation(
                                h_t[:], ps_u[:],
                                mybir.ActivationFunctionType.Relu, bias=bi_sb[:])
                        else:
                            h_self = (xE, xO)[arr]
                            h_other = (xO, xE)[arr]
                            msg = work.tile([D, SLAB], f32, tag=f"msg{arr}")
                            nc.vector.tensor_sub(msg[:], a_fm[:],
                                                 h_other[:, c0:c0 + SLAB])
                            nc.tensor.matmul(ps_u[:], Wu_sb[:], msg[:],
                                             start=True, stop=True)
                            tmp = work.tile([D, SLAB], f32, tag=f"tmp{arr}")
                            nc.vector.scalar_tensor_tensor(
                                tmp[:], ps_u[:], bu_sb[:],
                                h_self[:, c0:c0 + SLAB],
                                op0=mybir.AluOpType.add, op1=mybir.AluOpType.add)
                            h_t = work.tile([D, SLAB], f32, tag=f"ht{arr}")
                            nc.scalar.activation(
                                h_t[:], tmp[:], mybir.ActivationFunctionType.Relu)
                        nc.sync.dma_start(
                            hout[arr][:][:, bass.ds(i * group + off + c0, SLAB)],
                            h_t[:])
                        ps_e = (psA if arr == 0 else psB).tile(
                            [128, NB, D], f32, tag="em")
                        transpose_to_em(ps_e, h_t, NB)
                        em_t = (emE, emO)[arr]
                        nc.scalar.copy(em_t[:, sub * NB:(sub + 1) * NB, :],
                                       ps_e[:])
                # scatter: hE goes to dst half (a) by dIdx, hO to src half (b)
                nc.gpsimd.dma_scatter_add(
                    tab_slice(tgt, a), emE[:], idx_slice("d", i, off),
                    num_idxs=group, num_idxs_reg=group, elem_size=D,
                    single_packet=False)
                nc.gpsimd.dma_scatter_add(
                    tab_slice(tgt, b), emO[:], idx_slice("s", i, off),
                    num_idxs=group, num_idxs_reg=group, elem_size=D,
                    single_packet=False)

            def run_pass(kind, src, tgt, hin, hout):
                for seg in range(4):
                    n_groups = meta["seg_sz"][seg] // group
                    with tc.For_i(0, n_groups, unroll) as i:
                        for j in range(unroll):
                            body(kind, seg, i + j, src, tgt, hin, hout)

            def allreduce(src, dst):
                nc.gpsimd.collective_compute(
                    "AllReduce", mybir.AluOpType.add,
                    replica_groups=[list(range(n_cores))],
                    ins=[src.opt()], outs=[dst.opt()])

            # ---- schedule -----------------------------------------------
            # scatter targets per pass:  A C B A C ; AR outs: B A C B A
            tgt_seq = [0, 2, 1, 0, 2]
            ar_out = [1, 0, 2, 1, 0]
            zero_table(agg[0])
            zero_table(agg[2])
            run_pass("init", None, agg[0], None, (hE[0], hO[0]))
            allreduce(agg[0], agg[1])
            for p in range(1, STEPS + 1):
                tgt = agg[tgt_seq[p]]
                if p >= 2:
                    zero_table(tgt)
                run_pass("step", agg[ar_out[p - 1]], tgt,
                         (hE[(p + 1) % 2], hO[(p + 1) % 2]),
                         (hE[p % 2], hO[p % 2]))
                allreduce(tgt, agg[ar_out[p]])
            aggF = agg[ar_out[STEPS]]

            # ---- final: out = relu([nf || agg] @ Wf + bf) ----------------
            def final_body(j, roff):
                nf_em = emb.tile([128, NB, D], f32, tag="fnf")
                nc.sync.dma_start(
                    nf_em[:],
                    nfp_t.ap()[bass.ds(j * SLAB + roff, SLAB), :]
                    .rearrange("(a p) f -> p a f", p=128))
                ag_em = emb.tile([128, NB, D], f32, tag="fag")
                nc.sync.dma_start(
                    ag_em[:],
                    aggF[:][bass.ds(j * SLAB + roff, SLAB), :]
                    .rearrange("(a p) f -> p a f", p=128))
                ps_n = psA.tile([D, SLAB], f32, tag="afm")
                transpose_to_fm(ps_n, nf_em, NB, 0)
                nf_fm = work.tile([D, SLAB], f32, tag="fnm")
                nc.scalar.copy(nf_fm[:], ps_n[:])
                ps_g = psB.tile([D, SLAB], f32, tag="afm")
                transpose_to_fm(ps_g, ag_em, NB, 0)
                ag_fm = work.tile([D, SLAB], f32, tag="fgm")
                nc.scalar.copy(ag_fm[:], ps_g[:])
                ps_o = psA.tile([128, NB, D], f32, tag="em")
                for blk in range(NB):
                    nc.tensor.matmul(ps_o[:, blk, :],
                                     nf_fm[:, blk * 128:(blk + 1) * 128],
                                     WfA[:], start=True, stop=False)
                    nc.tensor.matmul(ps_o[:, blk, :],
                                     ag_fm[:, blk * 128:(blk + 1) * 128],
                                     WfB[:], start=False, stop=True)
                tmp = work.tile([128, NB, D], f32, tag="ftmp")
                nc.vector.tensor_add(
                    tmp[:], ps_o[:],
                    bf4_sb[:].rearrange("p (a f) -> p a f", f=D))
                o_t = work.tile([128, NB, D], f32, tag="fot")
                nc.scalar.activation(o_t[:], tmp[:],
                                     mybir.ActivationFunctionType.Relu)
                nc.sync.dma_start(
                    out_t.ap()[bass.ds(j * SLAB, SLAB), :]
                    .rearrange("(a p) f -> p a f", p=128),
                    o_t[:])

            with tc.For_i(0, jb, 1) as j:
                final_body(j, 0)
            with tc.For_i(jb, OUT_PAD // SLAB, 1) as j:
                final_body(j, PAD_HI)

    nc.compile()
    return nc


LAST_RESULTS = None
LAST_TIMES = None


def _run_spmd(nc, in_maps, time_iters=0):
    """Execute the bass module on len(in_maps) axon cores via PJRT.

    Mirrors bass2jax.run_bass_via_pjrt but without output donation (this
    kernel writes every output element), so the jitted callable can be
    re-executed for wall-clock timing.  Returns (per-core results, times).
    """
    import time as _time

    import jax
    from jax.experimental.shard_map import shard_map
    from jax.sharding import Mesh, NamedSharding, PartitionSpec

    from concourse import bass2jax, mybir

    bass2jax.install_neuronx_cc_hook()
    n_cores = len(in_maps)
    partition_name = (nc.partition_id_tensor.name
                      if nc.partition_id_tensor else None)
    in_names, out_names, out_avals, zero_outs = [], [], [], []
    for alloc in nc.m.functions[0].allocations:
        if not isinstance(alloc, mybir.MemoryLocationSet):
            continue
        name = alloc.memorylocations[0].name
        if alloc.kind == "ExternalInput":
            if name != partition_name:
                in_names.append(name)
        elif alloc.kind == "ExternalOutput":
            shape = tuple(alloc.tensor_shape)
            dtype = mybir.dt.np(alloc.dtype)
            out_names.append(name)
            out_avals.append(jax.core.ShapedArray(shape, dtype))
            zero_outs.append(np.zeros(shape, dtype))
    n_params = len(in_names)
    full_in_names = list(in_names) + list(out_names)
    if partition_name is not None:
        full_in_names.append(partition_name)

    def _body(*args):
        operands = list(args)
        if partition_name is not None:
            operands.append(bass2jax.partition_id_tensor())
        outs = bass2jax._bass_exec_p.bind(
            *operands,
            out_avals=tuple(out_avals),
            in_names=tuple(full_in_names),
            out_names=tuple(out_names),
            lowering_input_output_aliases=(),
            sim_require_finite=True,
            sim_require_nnan=True,
            nc=nc,
        )
        return tuple(outs)

    devices = jax.devices()[:n_cores]
    mesh = Mesh(np.asarray(devices), ("core",))
    spec = NamedSharding(mesh, PartitionSpec("core"))
    n_in = n_params + len(zero_outs)
    fn = jax.jit(shard_map(_body, mesh=mesh,
                           in_specs=(PartitionSpec("core"),) * n_in,
                           out_specs=(PartitionSpec("core"),) * len(out_names),
                           check_rep=False))
    dev_in = [
        jax.device_put(
            np.concatenate([np.asarray(in_maps[c][k]) for c in range(n_cores)], 0),
            spec)
        for k in in_names
    ]
    dev_zero = [
        jax.device_put(np.zeros((n_cores * z.shape[0], *z.shape[1:]), z.dtype), spec)
        for z in zero_outs
    ]
    out = fn(*dev_in, *dev_zero)
    jax.block_until_ready(out)
    times = []
    for _ in range(time_iters):
        t0 = _time.perf_counter()
        out2 = fn(*dev_in, *dev_zero)
        jax.block_until_ready(out2)
        times.append(_time.perf_counter() - t0)
    results = [
        {name: np.asarray(out[i]).reshape(n_cores, *out_avals[i].shape)[c]
         for i, name in enumerate(out_names)}
        for c in range(n_cores)
    ]
    return results, times


def kernel(node_feature, edge_feature, edge_src, edge_dst,
           Wi, bi, Wu, bu, Wf, bf):
    import os

    global LAST_RESULTS, LAST_TIMES
    shards, meta = _prep(node_feature, edge_feature, edge_src, edge_dst)
    nc = _build(meta)

    Wi = np.asarray(Wi, np.float32)
    Wu = np.asarray(Wu, np.float32)
    Wf = np.asarray(Wf, np.float32)
    bi = np.asarray(bi, np.float32)
    bu = np.asarray(bu, np.float32)
    bf = np.asarray(bf, np.float32)
    common = {
        "Wi": Wi, "Wu": Wu, "Wf": Wf,
        "bi": bi.reshape(D, 1), "bu": bu.reshape(D, 1),
        "bf4": np.tile(bf, (D, SLAB // D)).reshape(D, SLAB),
        "ident": np.eye(D, dtype=np.float32),
    }
    in_maps = [dict(sh, **common) for sh in shards]
    time_iters = int(os.environ.get("KERNEL_TIME_ITERS", "0"))
    results, times = _run_spmd(nc, in_maps, time_iters=time_iters)
    LAST_RESULTS = results
    LAST_TIMES = times
    return np.asarray(results[0]["out"][:meta["N"]])

